# revision 1
# baseline (speedup 1.0000x reference)
"""Trainium2 Bass kernel for nn_CompositeLoss (focal + sparsity + concentration).

Strategy (data-parallel over batch, 8 cores x 2 batch each = 40 images/core):
  One streaming pass over z=pred, t=target per core.
  Per element:  s = 2t-1 (fp16), w = s*z (fp32), pt = sigmoid(w) (fp32),
                L = ln(pt+1e-8) (fp16), d2 = (1-pt)^2 (fp16), G = d2*L (fp16),
                H' = (s<0)*G (fp16 = (1-t)*G)
  Reductions (free-dim accum on the producing op, fp32):
                SG=sum(G), SH=sum(H'), SW=sum(w), SA=sum|z|; bn_stats(z) gives
                sum(z), sum(z^2).  sum(z*t) = (SW + sum z)/2.
  Concentration via PE matmuls: per image-half, stationary [1, yh, yh^2]
  column weights give per-x-column moments of p=sigmoid(z) (fp32 matmul) and
  t (float32r matmul); host combines with xh weights in float64.
  Host finalize is tiny (float64) and exactly mirrors the reference algebra.
"""

import os
import sys
import numpy as np

sys.path.insert(0, "/opt/trn_rl_repo")

B, C, H, W = 16, 20, 256, 256
N_CORES = 8
B_PER_CORE = B // N_CORES            # 2
IMG_PER_CORE = B_PER_CORE * C        # 40
MB = 4                               # images per megabatch
NMB = IMG_PER_CORE // MB             # 10
FD = MB * 512                        # 2048 free elems per tile
NTOT = float(B * C * H * W)

ALPHA, GAMMA = 0.25, 2.0
SPARSITY_PENALTY = 1.0
FOCAL_W, SPARSITY_W, CONC_W = 1.0, 0.8, 1.5

_PROGRAM_CACHE = {}


def _build_program(reps=1, variant=None):
    variant = variant or os.environ.get("KVARIANT", "full")
    from contextlib import ExitStack
    import concourse.bass as bass
    import concourse.tile as tile
    import concourse.bacc as bacc
    from concourse import mybir

    dt = mybir.dt
    Act = mybir.ActivationFunctionType
    Alu = mybir.AluOpType

    nc = bacc.Bacc("TRN2", target_bir_lowering=False, debug=False,
                   num_devices=N_CORES)

    z_d = nc.dram_tensor("z", [128, IMG_PER_CORE * 2 * 256], dt.float32,
                         kind="ExternalInput").ap()
    t_d = nc.dram_tensor("t", [128, IMG_PER_CORE * 2 * 256], dt.float32,
                         kind="ExternalInput").ap()
    w_d = nc.dram_tensor("wts", [128, 6], dt.float32,
                         kind="ExternalInput").ap()
    w16_d = nc.dram_tensor("wts16", [128, 6], dt.float16,
                           kind="ExternalInput").ap()
    moms_d = nc.dram_tensor("moms", [NMB, 3, 2 * MB * 256], dt.float32,
                            kind="ExternalOutput").ap()
    acc_d = nc.dram_tensor("acc", [128, NMB, 6], dt.float32,
                           kind="ExternalOutput").ap()

    do_act = variant in ("full", "act")
    do_dve = variant in ("full", "dve")
    do_pe = variant in ("full", "pe")

    with tile.TileContext(nc) as tc, ExitStack() as ctx:
        io_pool = ctx.enter_context(tc.tile_pool(name="io", bufs=2))
        f32_pool = ctx.enter_context(tc.tile_pool(name="f32", bufs=2))
        f16_pool = ctx.enter_context(tc.tile_pool(name="f16", bufs=2))
        psum_pool = ctx.enter_context(
            tc.tile_pool(name="psum", bufs=2, space="PSUM"))
        const_pool = ctx.enter_context(tc.tile_pool(name="const", bufs=1))
        stat_pool = ctx.enter_context(tc.tile_pool(name="stat", bufs=1))

        wt = const_pool.tile([128, 6], dt.float32, tag="wts")
        nc.sync.dma_start(wt[:], w_d[:])
        wt16 = const_pool.tile([128, 6], dt.float16, tag="wts16")
        nc.sync.dma_start(wt16[:], w16_d[:])

        eps_t = const_pool.tile([128, 1], dt.float32, tag="eps")
        nc.gpsimd.memset(eps_t[:], 1e-8)

        accs = stat_pool.tile([128, NMB, 6], dt.float32, tag="accs")
        wrote_accs = False
        wrote_moms = False

        for _ in range(reps):
            for b in range(NMB):
                z_t = io_pool.tile([128, FD], dt.float32, tag="z")
                nc.sync.dma_start(z_t[:], z_d[:, b * FD:(b + 1) * FD])
                t_t = io_pool.tile([128, FD], dt.float32, tag="t")
                nc.sync.dma_start(t_t[:], t_d[:, b * FD:(b + 1) * FD])
                if variant == "dma":
                    continue

                # s = 2t-1 in fp16 (exact +-1)
                s_t = f16_pool.tile([128, FD], dt.float16, tag="s")
                if do_dve or do_pe:
                    nc.vector.tensor_scalar(s_t[:], t_t[:], 2.0, -1.0,
                                            Alu.mult, Alu.add)

                if do_dve:
                    # w = s*z fp32, accum -> sum(w)
                    w_t = f32_pool.tile([128, FD], dt.float32, tag="w")
                    nc.vector.scalar_tensor_tensor(
                        w_t[:], s_t[:], 0.0, z_t[:], Alu.bypass, Alu.mult,
                        accum_out=accs[:, b, 0:1])
                    wrote_accs = True

                if do_act:
                    src_w = w_t if variant == "full" else z_t
                    pt_t = f32_pool.tile([128, FD], dt.float32, tag="pt")
                    nc.scalar.activation(pt_t[:], src_w[:], Act.Sigmoid)
                    L_t = f16_pool.tile([128, FD], dt.float16, tag="L")
                    nc.scalar.activation(L_t[:], pt_t[:], Act.Ln,
                                         bias=eps_t[:])
                    d2_t = f16_pool.tile([128, FD], dt.float16, tag="d2")
                    nc.scalar.activation(d2_t[:], pt_t[:], Act.Square,
                                         scale=-1.0, bias=1.0)
                if do_act or do_pe:
                    p_t = f32_pool.tile([128, FD], dt.float32, tag="p")
                    nc.scalar.activation(p_t[:], z_t[:], Act.Sigmoid)
                if do_act:
                    zq_t = f16_pool.tile([128, FD], dt.float16, tag="scr4")
                    nc.scalar.activation(zq_t[:], z_t[:], Act.Square,
                                         accum_out=accs[:, b, 5:6])
                    wrote_accs = True

                if do_dve:
                    ga = L_t if variant == "full" else s_t
                    gb = d2_t if variant == "full" else s_t
                    # G = d2*L fp16, accum -> sum(G)
                    G_t = f16_pool.tile([128, FD], dt.float16, tag="G")
                    nc.vector.scalar_tensor_tensor(
                        G_t[:], gb[:], 0.0, ga[:], Alu.bypass, Alu.mult,
                        accum_out=accs[:, b, 1:2])
                    # H' = (s<0)*G fp16 = (1-t)*G
                    H_t = f16_pool.tile([128, FD], dt.float16, tag="scr")
                    nc.vector.scalar_tensor_tensor(
                        H_t[:], s_t[:], 0.0, G_t[:], Alu.is_lt, Alu.mult,
                        accum_out=accs[:, b, 2:3])
                    # sum |z| and sum z
                    nc.vector.tensor_reduce(
                        accs[:, b, 3:4], z_t[:], mybir.AxisListType.X,
                        Alu.add, apply_absolute_value=True)
                    nc.vector.tensor_reduce(
                        accs[:, b, 4:5], z_t[:], mybir.AxisListType.X,
                        Alu.add)

                if do_pe:
                    # PE moments: p-moms cols 0:1024, s-moms cols 1024:2048
                    pm = psum_pool.tile([3, 2 * MB * 256], dt.float32,
                                        tag="pm")
                    for i in range(MB):
                        for h in range(2):
                            sl = slice(i * 512 + h * 256,
                                       i * 512 + (h + 1) * 256)
                            osl = slice(i * 256, (i + 1) * 256)
                            tsl = slice(MB * 256 + i * 256,
                                        MB * 256 + (i + 1) * 256)
                            nc.tensor.matmul(
                                pm[:, osl], wt[:, h * 3:h * 3 + 3],
                                p_t[:, sl], start=(h == 0), stop=(h == 1))
                            nc.tensor.matmul(
                                pm[0:2, tsl], wt16[:, h * 3:h * 3 + 2],
                                s_t[:, sl], start=(h == 0), stop=(h == 1))
                    mom_sb = f32_pool.tile([3, 2 * MB * 256], dt.float32,
                                           tag="momsb")
                    nc.scalar.copy(mom_sb[:, 0:MB * 256], pm[:, 0:MB * 256])
                    nc.vector.tensor_copy(mom_sb[0:2, MB * 256:],
                                          pm[0:2, MB * 256:])
                    nc.sync.dma_start(moms_d[b], mom_sb[:])
                    wrote_moms = True

        if wrote_accs:
            nc.sync.dma_start(acc_d[:], accs[:])

    nc.compile()
    return nc


def _get_program(reps=1):
    key = (reps, os.environ.get("KVARIANT", "full"))
    if key not in _PROGRAM_CACHE:
        _PROGRAM_CACHE[key] = _build_program(reps)
    return _PROGRAM_CACHE[key]


def _host_inputs(pred, target):
    """Build per-core input maps (partition-major layout + coord weights)."""
    yh_top = np.arange(128, dtype=np.float64) - 127.5
    yh_bot = np.arange(128, dtype=np.float64) + 0.5
    wts = np.stack([np.ones(128), yh_top, yh_top * yh_top,
                    np.ones(128), yh_bot, yh_bot * yh_bot],
                   axis=1).astype(np.float32)
    wts16 = wts.astype(np.float16)
    wts16[:, 2] = 0
    wts16[:, 5] = 0

    in_maps = []
    for c in range(N_CORES):
        b0 = c * B_PER_CORE
        z = pred[b0:b0 + B_PER_CORE].reshape(IMG_PER_CORE, 2, 128, 256)
        t = target[b0:b0 + B_PER_CORE].reshape(IMG_PER_CORE, 2, 128, 256)
        z = np.ascontiguousarray(z.transpose(2, 0, 1, 3)).reshape(128, -1)
        t = np.ascontiguousarray(t.transpose(2, 0, 1, 3)).reshape(128, -1)
        in_maps.append({"z": z, "t": t, "wts": wts, "wts16": wts16})
    return in_maps


def _finalize(results):
    """Combine per-core outputs into the 4 loss scalars (float64 host math)."""
    SG = SH = SW = SA = Sz = Sz2 = 0.0
    pm_all = []
    tm_all = []
    for r in results:
        acc = r["acc"].astype(np.float64)        # [128, NMB, 6]
        moms = r["moms"].astype(np.float64)      # [NMB, 5, MB*256]
        SW += acc[..., 0].sum()
        SG += acc[..., 1].sum()
        SH += acc[..., 2].sum()
        SA += acc[..., 3].sum()
        Sz += acc[..., 4].sum()
        Sz2 += acc[..., 5].sum()
        pm = moms[:, 0:3, 0:MB * 256].reshape(NMB, 3, MB, 256)
        sm = moms[:, 0:2, MB * 256:].reshape(NMB, 2, MB, 256)
        tm = np.empty_like(sm)
        tm[:, 0] = (sm[:, 0] + 256.0) / 2.0   # sum t = (sum s + sum 1)/2
        tm[:, 1] = sm[:, 1] / 2.0             # sum t*yh = sum s*yh / 2
        pm_all.append(pm.transpose(0, 2, 1, 3).reshape(IMG_PER_CORE, 3, 256))
        tm_all.append(tm.transpose(0, 2, 1, 3).reshape(IMG_PER_CORE, 2, 256))

    pm = np.concatenate(pm_all, 0)   # [320, 3, 256]
    tm = np.concatenate(tm_all, 0)   # [320, 2, 256]

    focal = -(0.25 * SG + 0.5 * SH) / NTOT
    Szt = (SW + Sz) / 2.0
    T0_img = tm[:, 0, :].sum(1)
    sparsity = (Sz2 - 2.0 * Szt + T0_img.sum()) / NTOT \
        + SPARSITY_PENALTY * SA / NTOT

    xh = np.arange(W, dtype=np.float64) - 127.5
    Ty = tm[:, 1, :].sum(1)
    Tx = (tm[:, 0, :] * xh).sum(1)
    P0 = pm[:, 0, :].sum(1)
    Py = pm[:, 1, :].sum(1)
    Px = (pm[:, 0, :] * xh).sum(1)
    Pr = pm[:, 2, :].sum(1) + (pm[:, 0, :] * xh * xh).sum(1)

    valid = T0_img > 0
    safe = np.where(valid, T0_img, 1.0)
    cy = Ty / safe
    cx = Tx / safe
    per = (Pr - 2 * cy * Py - 2 * cx * Px + (cy * cy + cx * cx) * P0) \
        / float(H * W)
    nv = int(valid.sum())
    conc = (np.where(valid, per, 0.0).sum() / max(nv, 1)) if nv > 0 else 0.0

    total = FOCAL_W * focal + SPARSITY_W * sparsity + CONC_W * conc
    return (np.float32(total), np.float32(focal), np.float32(sparsity),
            np.float32(conc))


def _run(in_maps, reps=1, trace=False):
    from concourse.bass_utils import run_bass_kernel_spmd
    nc = _get_program(reps)
    last_err = None
    for _ in range(3):
        try:
            return run_bass_kernel_spmd(nc, in_maps, list(range(N_CORES)),
                                        trace=trace)
        except Exception as e:  # transient device errors happen; retry
            last_err = e
    raise last_err


def kernel(pred, target):
    pred = np.ascontiguousarray(pred, dtype=np.float32)
    target = np.ascontiguousarray(target, dtype=np.float32)
    in_maps = _host_inputs(pred, target)
    res = _run(in_maps, reps=int(os.environ.get("KERNEL_REPS", "1")))
    return _finalize(res.results)

